# revision 27
# baseline (speedup 1.0000x reference)
"""Sparse (class-gated bilinear) attention kernel for TRN2, 8 NeuronCores.

Problem shapes (hardcoded): b=2, h=8, s=512, d=64, C=8 classes, B=4 bases.

Per (b,h), with scores laid out [j (key, partitions), i (query, free)]:
  UT_c[n,i] = sum_m (W1e[c][m,n]/sqrt(d)) * Q[i,m]               (host)
  ST_c[j,i] = sum_n K[j,n] * UT_c[n,i]                           (PE, f32r)
  exp-path classes c in {2,3,6,7}:
    PSUM bank pre-filled with Bneg_c = -57344*(b_mat^T != c) via an
    identity matmul, the ST matmul accumulates on top, and one ACT exp
    yields the *already masked* eb_c = exp(ST_c)*m_c. ec_c = eb_c*erp.
  chain classes {0,1,4,5}:
    in-place copy_predicated chain (uint8 host masks) selects
    sel = ST_{bmat} among them; et = exp(sel)*erp; ec_c = et*m_c.
  Telescoped output (c=0 needs no mask):
    out = E (x) t_0 + sum_{c>=1} ec_c (x) (t_c - t_0),  E = et
  t carries a ones column (t_0 only) so Z = sum_j E accumulates free.
  Output matmuls run with ec as the *stationary* operand and t [128,65]
  moving -> only 65 PE rows per matmul.
  out[i,D] = oacc[i,D] / oacc[i,64]                              (host)

Sharding: 16 (b,h) pairs over 8 cores; core k handles b=k//4,
heads (2*(k%4), 2*(k%4)+1). Masks shared by both heads of a core.
"""

import sys

import numpy as np

if "/opt/trn_rl_repo" not in sys.path:
    sys.path.insert(0, "/opt/trn_rl_repo")

import ml_dtypes

B_, H_, S_, D_, C_ = 2, 8, 512, 64, 8
NCORES = 8
JT = S_ // 128  # 4 j-tiles

EXP_CLASSES = (2, 3, 6, 7)  # per-class exp pairs with Bneg prefill
EXP_SINGLE = (1,)  # additional exp-path class (single)
CHAIN_PREDS = (4, 5)  # chain classes minus the base (0)
BIGNEG = -57344.0  # exactly representable in bf16; exp(x+BIGNEG) == 0

_CACHE = {}


def _softmax(a, axis):
    e = np.exp(a - a.max(axis=axis, keepdims=True))
    return e / e.sum(axis=axis, keepdims=True)


def _build_nc():
    import concourse.bass as bass  # noqa: F401
    import concourse.mybir as mybir
    from concourse import bacc
    from concourse.tile import TileContext

    f32 = mybir.dt.float32
    f32r = mybir.dt.float32r
    bf16 = mybir.dt.bfloat16
    fp8 = mybir.dt.float8e5
    f16 = mybir.dt.float16
    u8 = mybir.dt.uint8

    nc = bacc.Bacc("TRN2", target_bir_lowering=False, debug=False)

    kt_d = nc.dram_tensor("kt", [2, 64, 512], f16, kind="ExternalInput").ap()
    ut_d = nc.dram_tensor("ut", [2, 2, 64, 2048], f16, kind="ExternalInput").ap()
    # t~: [128, jt*520 + c*65 + d]; c=0 plane is t_0 (ones col), c>=1 are
    # t_c - t_0 (zero col 64)
    t_d = nc.dram_tensor("t", [2, 128, 2080], bf16, kind="ExternalInput").ap()
    erp_d = nc.dram_tensor("erp", [2, 128, 2048], bf16, kind="ExternalInput").ap()
    bt_d = nc.dram_tensor("bt", [128, 2048], bf16, kind="ExternalInput").ap()
    idn_d = nc.dram_tensor("idn", [128, 128], fp8, kind="ExternalInput").ap()
    # uint8 pred masks for the chain classes, per j-tile
    prd_d = nc.dram_tensor(
        "prd", [4, 128, len(CHAIN_PREDS) * 512], u8, kind="ExternalInput"
    ).ap()
    # bf16 BIGNEG planes per j-tile: one per exp-path class (bt != c), plus
    # a sel-plane (bt in EXP_CLASSES) so the chain's et is exactly E*m_chain
    bng_d = nc.dram_tensor(
        "bng", [4, 128, (len(EXP_CLASSES) + 2) * 512], fp8, kind="ExternalInput"
    ).ap()
    ot_d = nc.dram_tensor("ot", [2, 128, 260], f32, kind="ExternalOutput").ap()

    EXP = mybir.ActivationFunctionType.Exp
    EQ = mybir.AluOpType.is_equal

    with TileContext(nc) as tc:
        with (
            tc.tile_pool(name="inp", bufs=1) as ipool,
            tc.tile_pool(name="msk", bufs=1) as mpool,
            tc.tile_pool(name="work", bufs=4) as wpool,
            tc.tile_pool(name="ec", bufs=8) as epool,
            tc.tile_pool(name="pst", bufs=4, space="PSUM") as pst,
            tc.tile_pool(name="psel", bufs=2, space="PSUM") as psel,
            tc.tile_pool(name="pacc", bufs=1, space="PSUM") as pacc,
        ):
            kt = ipool.tile([64, 1024], f16, tag="kt", name="kt")
            idn = ipool.tile([128, 128], fp8, tag="idn", name="idn")
            bts = ipool.tile([128, 2048], bf16, tag="bt", name="bt")
            ut, erp, tsb = {}, {}, {}
            for p in range(2):
                ut[p] = ipool.tile([64, 4096], f16, tag=f"ut{p}", name=f"ut{p}")
                erp[p] = ipool.tile([128, 2048], bf16, tag=f"erp{p}", name=f"erp{p}")
                tsb[p] = ipool.tile([128, 2080], bf16, tag=f"t{p}", name=f"t{p}")
            prd, bng = [], []
            for jt in range(JT):
                prd.append(
                    ipool.tile(
                        [128, len(CHAIN_PREDS) * 512], u8, tag=f"pr{jt}", name=f"pr{jt}"
                    )
                )
                bng.append(
                    ipool.tile(
                        [128, (len(EXP_CLASSES) + 2) * 512], fp8, tag=f"bn{jt}",
                        name=f"bn{jt}",
                    )
                )

            # DMAs ordered by first use, smallest-first at the head
            nc.sync.dma_start(out=kt[:, :512], in_=kt_d[0])
            nc.sync.dma_start(out=idn, in_=idn_d)
            nc.sync.dma_start(out=bng[0], in_=bng_d[0])
            nc.sync.dma_start(out=ut[0][:, :2048], in_=ut_d[0, 0])
            nc.sync.dma_start(out=ut[0][:, 2048:], in_=ut_d[0, 1])
            nc.sync.dma_start(out=prd[0], in_=prd_d[0])
            nc.sync.dma_start(out=bts, in_=bt_d)
            nc.sync.dma_start(out=erp[0], in_=erp_d[0])
            nc.sync.dma_start(out=tsb[0], in_=t_d[0])
            nc.sync.dma_start(out=kt[:, 512:], in_=kt_d[1])
            nc.sync.dma_start(out=ut[1][:, :2048], in_=ut_d[1, 0])
            nc.sync.dma_start(out=erp[1], in_=erp_d[1])
            nc.sync.dma_start(out=ut[1][:, 2048:], in_=ut_d[1, 1])
            nc.sync.dma_start(out=tsb[1], in_=t_d[1])
            nc.sync.dma_start(out=bng[1], in_=bng_d[1])
            nc.sync.dma_start(out=prd[1], in_=prd_d[1])
            nc.sync.dma_start(out=bng[2], in_=bng_d[2])
            nc.sync.dma_start(out=prd[2], in_=prd_d[2])
            nc.sync.dma_start(out=bng[3], in_=bng_d[3])
            nc.sync.dma_start(out=prd[3], in_=prd_d[3])

            # per-jt bf16 ec masks for chain classes 1,4,5 (built just in
            # time inside the loop so they don't block the first preds)
            m145 = [None] * JT

            def build_masks(jt):
                bslice = bts[:, jt * 512 : (jt + 1) * 512]
                mp = mpool.tile([128, 1024], bf16, tag=f"m45_{jt}", name=f"m45_{jt}")
                nc.vector.tensor_scalar(mp[:, :512], bslice, 4.0, None, EQ)
                nc.vector.tensor_scalar(mp[:, 512:], bslice, 5.0, None, EQ)
                m145[jt] = mp

            oacc = {}
            for p in range(2):
                oacc[p] = pacc.tile([128, 260], f32, tag=f"o{p}", name=f"oacc{p}")

            pending = None

            def emit_muls_and_flush():
                eb23_, eb67_, eb1_, eraw_, p_, jt_ = pending
                erps = erp[p_][:, jt_ * 512 : (jt_ + 1) * 512]
                erpb = erps[:, None, :].to_broadcast([128, 2, 512])
                et = epool.tile([128, 512], bf16, tag="et", name="et")
                nc.vector.tensor_mul(et, eraw_, erps)
                ec23 = epool.tile([128, 1024], bf16, tag="ec23", name="ec23")
                nc.vector.tensor_mul(
                    ec23.rearrange("p (two f) -> p two f", two=2),
                    eb23_.rearrange("p (two f) -> p two f", two=2),
                    erpb,
                )
                ec67 = epool.tile([128, 1024], bf16, tag="ec67", name="ec67")
                nc.gpsimd.tensor_mul(
                    ec67.rearrange("p (two f) -> p two f", two=2),
                    eb67_.rearrange("p (two f) -> p two f", two=2),
                    erpb,
                )
                ec1 = epool.tile([128, 512], bf16, tag="ec1", name="ec1")
                nc.vector.tensor_mul(ec1, eb1_, erps)
                ec45 = epool.tile([128, 1024], bf16, tag="ec45", name="ec45")
                nc.vector.tensor_mul(
                    ec45.rearrange("p (two f) -> p two f", two=2),
                    et[:, None, :].to_broadcast([128, 2, 512]),
                    m145[jt_].rearrange("p (two f) -> p two f", two=2),
                )
                srcs_ = {
                    0: (et, 0),
                    1: (ec1, 0),
                    2: (ec23, 0),
                    3: (ec23, 512),
                    4: (ec45, 0),
                    5: (ec45, 512),
                    6: (ec67, 0),
                    7: (ec67, 512),
                }
                for c in range(C_):
                    src, off = srcs_[c]
                    for ic in range(4):
                        # PSUM pending-zero granularity is 2KB x touched
                        # partitions: exactly ONE start per oacc bank, the
                        # other ic regions' first writes clear their own
                        # pending bytes and accumulate thereafter.
                        nc.tensor.matmul(
                            oacc[p_][:, ic * 65 : (ic + 1) * 65],
                            src[:, off + ic * 128 : off + (ic + 1) * 128],
                            tsb[p_][:, jt_ * 520 + c * 65 : jt_ * 520 + (c + 1) * 65],
                            start=(jt_ == 0 and c == 0 and ic == 0),
                            stop=(jt_ == JT - 1 and c == C_ - 1),
                            skip_group_check=True,
                        )

            for jt in range(JT):
                build_masks(jt)
                for p in range(2):
                    sel = psel.tile([128, 512], f32, tag="sel")
                    sp = {}
                    # chain classes first so the chain starts early. The sel
                    # bank is pre-filled with BIGNEG at exp-path positions so
                    # et comes out already masked to the chain classes.
                    nsel = len(EXP_CLASSES) + 1
                    nc.tensor.matmul(
                        sel,
                        idn,
                        bng[jt][:, nsel * 512 : (nsel + 1) * 512],
                        start=True,
                        stop=False,
                    )
                    nc.tensor.matmul(
                        sel,
                        kt[:, p * 512 + jt * 128 : p * 512 + (jt + 1) * 128],
                        ut[p][:, 0:512],
                        start=False,
                        stop=True,
                    )
                    for c in (4, 5):
                        dst = pst.tile([128, 512], f32, tag="st")
                        nc.tensor.matmul(
                            dst,
                            kt[:, p * 512 + jt * 128 : p * 512 + (jt + 1) * 128],
                            ut[p][:, c * 512 : (c + 1) * 512],
                            start=True,
                            stop=True,
                        )
                        sp[c] = dst
                    # class 1: exp-path single (bng plane index 4)
                    dst = pst.tile([128, 512], f32, tag="st")
                    nc.tensor.matmul(
                        dst, idn, bng[jt][:, 4 * 512 : 5 * 512],
                        start=True, stop=False,
                    )
                    nc.tensor.matmul(
                        dst,
                        kt[:, p * 512 + jt * 128 : p * 512 + (jt + 1) * 128],
                        ut[p][:, 512:1024],
                        start=False, stop=True,
                    )
                    sp[1] = dst
                    # exp-path classes: Bneg prefill, then accumulate the ST
                    for k, c in enumerate(EXP_CLASSES):
                        dst = pst.tile([128, 512], f32, tag="st")
                        nc.tensor.matmul(
                            dst,
                            idn,
                            bng[jt][:, k * 512 : (k + 1) * 512],
                            start=True,
                            stop=False,
                        )
                        nc.tensor.matmul(
                            dst,
                            kt[:, p * 512 + jt * 128 : p * 512 + (jt + 1) * 128],
                            ut[p][:, c * 512 : (c + 1) * 512],
                            start=False,
                            stop=True,
                        )
                        sp[c] = dst

                    # chain (DVE): sel <- ST_c where bt == c
                    for k, c in enumerate(CHAIN_PREDS):
                        nc.vector.copy_predicated(
                            sel, prd[jt][:, k * 512 : (k + 1) * 512], sp[c]
                        )

                    # exps (ACT): masked exp-path planes + the selected plane
                    eb23 = wpool.tile([128, 1024], bf16, tag="eb23")
                    nc.scalar.activation(eb23[:, :512], sp[2], EXP)
                    nc.scalar.activation(eb23[:, 512:], sp[3], EXP)
                    eb67 = wpool.tile([128, 1024], bf16, tag="eb67")
                    nc.scalar.activation(eb67[:, :512], sp[6], EXP)
                    nc.scalar.activation(eb67[:, 512:], sp[7], EXP)
                    eb1 = wpool.tile([128, 512], bf16, tag="eb1")
                    nc.scalar.activation(eb1, sp[1], EXP)
                    eraw = wpool.tile([128, 512], bf16, tag="eraw")
                    nc.scalar.activation(eraw, sel, EXP)

                    # E-muls + output matmuls for the PREVIOUS step: they are
                    # ready now, so the DVE pipeline keeps draining while this
                    # step's exps run on ACT.
                    if pending is not None:
                        emit_muls_and_flush()
                    pending = (eb23, eb67, eb1, eraw, p, jt)
                    if jt == JT - 1 and p == 1:
                        # head 0's accumulation just finished flushing
                        os0 = wpool.tile([128, 260], f32, tag="os")
                        nc.scalar.copy(os0, oacc[0])
                        nc.sync.dma_start(out=ot_d[0], in_=os0)
            emit_muls_and_flush()

            os1 = wpool.tile([128, 260], f32, tag="os")
            nc.scalar.copy(os1, oacc[1])
            nc.sync.dma_start(out=ot_d[1], in_=os1)

    nc.compile()
    return nc


def _get_nc():
    if "nc" not in _CACHE:
        _CACHE["nc"] = _build_nc()
    return _CACHE["nc"]


def kernel(**inputs):
    q = np.asarray(inputs["query"], np.float32)
    k = np.asarray(inputs["key"], np.float32)
    v = np.asarray(inputs["value"], np.float32)
    bm = np.asarray(inputs["b_mat"])
    rpb = np.asarray(inputs["rpb"], np.float32)
    W1 = np.asarray(inputs["W1"], np.float32)
    a1 = np.asarray(inputs["alpha1"], np.float32)
    W2 = np.asarray(inputs["W2"], np.float32)
    a2 = np.asarray(inputs["alpha2"], np.float32)
    mask = np.asarray(inputs["mask"])

    W1e = np.einsum("Bhmn,CBh->Chmn", W1, _softmax(a1, 1)) / np.sqrt(D_)
    W2e = np.einsum("BhdD,CBh->ChdD", W2, _softmax(a2, 1))

    # additive -inf pair mask would go here; spec guarantees mask == ones
    assert mask.all(), "kernel assumes all-ones mask (spec fill=ones)"

    idn = np.eye(128, dtype=ml_dtypes.float8_e5m2)

    in_maps = []
    for cid in range(NCORES):
        b = cid // 4
        hs = [2 * (cid % 4), 2 * (cid % 4) + 1]
        kt = np.stack([k[b, h].T for h in hs]).astype(np.float16)  # [2,64,512]
        ut = np.empty((2, 2, 64, 2048), np.float16)
        for pi, h in enumerate(hs):
            u = np.einsum("Cmn,im->Cni", W1e[:, h], q[b, h])  # [8,64,512]
            # sbuf layout [n, (c, i)], DMA'd in two class-halves
            un = u.transpose(1, 0, 2).reshape(64, 2, 2048)  # [n, half, (c,i)]
            ut[pi] = un.transpose(1, 0, 2)
        # telescoped t~ with ones column on c=0 only
        t = np.empty((2, 128, 2080), ml_dtypes.bfloat16)
        for pi, h in enumerate(hs):
            tc = np.einsum("jd,CdD->CjD", v[b, h], W2e[:, h])  # [8,512,64]
            tt = np.empty((S_, C_, 65), np.float32)
            tt[:, 0, :64] = tc[0].astype(np.float32)
            tt[:, 0, 64] = 1.0
            for c in range(1, C_):
                if c in EXP_CLASSES + EXP_SINGLE:
                    tt[:, c, :64] = tc[c].astype(np.float32)
                    tt[:, c, 64] = 1.0
                else:
                    tt[:, c, :64] = (tc[c] - tc[0]).astype(np.float32)
                    tt[:, c, 64] = 0.0
            t[pi] = (
                tt.reshape(4, 128, C_ * 65).transpose(1, 0, 2).reshape(128, 2080)
            ).astype(ml_dtypes.bfloat16)
        erp = np.empty((2, 128, 2048), ml_dtypes.bfloat16)
        for pi, h in enumerate(hs):
            e = np.exp(rpb[b, h]).T  # [j, i]
            erp[pi] = (
                e.reshape(4, 128, 512).transpose(1, 0, 2).reshape(128, 2048)
            ).astype(ml_dtypes.bfloat16)
        btj = bm[b].T.reshape(4, 128, 512)  # [jt, j, i]
        bt = (
            btj.transpose(1, 0, 2).reshape(128, 2048).astype(np.float32)
        ).astype(ml_dtypes.bfloat16)
        prd = np.empty((4, 128, len(CHAIN_PREDS) * 512), np.uint8)
        bng = np.empty((4, 128, (len(EXP_CLASSES) + 2) * 512), np.float32)
        for jt in range(4):
            for ki, c in enumerate(CHAIN_PREDS):
                prd[jt][:, ki * 512 : (ki + 1) * 512] = btj[jt] == c
            for ki, c in enumerate(EXP_CLASSES):
                bng[jt][:, ki * 512 : (ki + 1) * 512] = np.where(
                    btj[jt] == c, 0.0, BIGNEG
                )
            bng[jt][:, 4 * 512 : 5 * 512] = np.where(
                btj[jt] == 1, 0.0, BIGNEG
            )
            bng[jt][:, 5 * 512 : 6 * 512] = np.where(
                np.isin(btj[jt], EXP_CLASSES + EXP_SINGLE), BIGNEG, 0.0
            )
        in_maps.append(
            {
                "kt": kt,
                "ut": ut,
                "t": t,
                "erp": erp,
                "bt": bt,
                "idn": idn,
                "prd": prd,
                "bng": bng.astype(ml_dtypes.float8_e5m2),
            }
        )

    import time

    from concourse.bass_utils import run_bass_kernel_spmd

    try:
        res = run_bass_kernel_spmd(
            _get_nc(), in_maps, core_ids=list(range(NCORES))
        )
    except Exception:
        # transient NRT_EXEC_UNIT_UNRECOVERABLE from a previously wedged
        # device clears on redispatch
        time.sleep(5)
        res = run_bass_kernel_spmd(
            _get_nc(), in_maps, core_ids=list(range(NCORES))
        )
    _CACHE["last_res"] = res
    outs = res.results

    out = np.zeros((B_, H_, S_, D_), np.float32)
    for cid in range(NCORES):
        b = cid // 4
        hs = [2 * (cid % 4), 2 * (cid % 4) + 1]
        for pi, h in enumerate(hs):
            ot = np.asarray(outs[cid]["ot"][pi], np.float32)  # [128, 260]
            ot = ot.reshape(128, 4, 65)
            full = ot.transpose(1, 0, 2).reshape(512, 65)  # i = ic*128 + p
            out[b, h] = full[:, :64] / full[:, 64:65]
    return out
